# revision 10
# baseline (speedup 1.0000x reference)
"""DCN (DLRM-style deep & cross network) Trainium2 Bass kernel.

Sharding: data-parallel over batch across 8 NeuronCores (2048 samples/core).
Embedding tables (bf16) + MLP weights are replicated to every core's HBM.

Per-core pipeline (activations kept feature-major, i.e. transposed, for PE):
  1. Embedding gather via InstDMAGatherAnt: one instruction per
     (phase, category), fetching 256B quad-rows (4 bf16 vocab rows, the
     minimum 256B element) with int16 indices v//4, spread across all 4
     SWDGE queues.  Measured floor is ~2.5ns/descriptor (marginal) +
     ~1.15us fixed per instruction, so phases use the largest legal
     nidx (1024) up front and a small 256 tail: [1024, 768, 256].
  2. gidx is uploaded as a single [16, cols] tensor and replicated to
     the 8 16-partition groups by on-chip DMAs so the first gather
     launches within a few us (no 850KB serial upload in front).
  3. scalar copy + 3x copy_predicated (DVE) select the right 32-value
     quarter per lookup into the combined feature tile (no DMA-engine
     traffic besides the gathers themselves).
  4. PE 128x128 transposes -> ct[k] = combined^T chunks [128, <=512].
  5. MLP: h^T = relu(W^T @ x^T) chains, bf16 matmuls, fp32 accumulate.
  6. CrossNet folds to 4 packed dot products against x0 (alpha0..2,
     Wc_x) plus a scalar chain with host-precomputed alpha_l.b_j
     constants; final = sigmoid(x-part + Wc_h . h3 + bc).
"""

import numpy as np

import concourse.bass as bass
import concourse.mybir as mybir
import concourse.tile as tile
from concourse import bacc
from concourse.bass import broadcast_tensor_aps
from concourse.bass_utils import run_bass_kernel_spmd
F32 = mybir.dt.float32
BF16 = mybir.dt.bfloat16
I32 = mybir.dt.int32
I16 = mybir.dt.int16
I8 = mybir.dt.int8

B = 16384
NCORES = 8
BC = B // NCORES            # 2048 samples per core
NCAT = 26
VOCAB = 100000
EMB = 32
NNUM = 13
D = NCAT * EMB + NNUM       # 845

L1, L2, L3 = 1024, 512, 256
NCROSS = 3
KC = 7                      # feature chunks of 128 (6*128 + 77)
KW = [128] * 6 + [D - 6 * 128]
CPB = 128 // EMB            # 4 categories per 128-feature block
M1, M2, M3 = L1 // 128, L2 // 128, L3 // 128   # 8, 4, 2
NTILE = BC // 128           # 16 batch tiles per core

# gather phases: (first tile, #tiles); nidx = #tiles * 128 (max 1024).
# Front-loaded 1024 phase amortizes the ~1.15us/instruction fixed cost;
# the small late phases keep the post-gather compute tail short.
PHASES = [(0, 8), (8, 4), (12, 2), (14, 2)]
NPH = len(PHASES)
# compute chunks: (first tile, #tiles, phase fed by)
CHUNKS = [(0, 4, 0), (4, 4, 0), (8, 4, 1), (12, 2, 2), (14, 2, 3)]
NCHUNK = len(CHUNKS)
# idx column offset (in 16-partition-wrapped cols) per phase block
_PHOFF = [0]
for _t0, _nt in PHASES:
    _PHOFF.append(_PHOFF[-1] + NCAT * _nt * 8)
GIDX_COLS = _PHOFF[-1]      # 3328 total
_P0COLS = _PHOFF[1]         # phase-0 block


def _build(cross_consts, queue_map=None) -> bass.Bass:
    # cross_consts = (c10, c20, c21, d0, d1, d2):
    #   c_lj = alpha_l . cross_bias_j,  d_j = Wc_x . cross_bias_j
    c10, c20, c21, d0, d1, d2 = cross_consts

    nc = bacc.Bacc("TRN2", target_bir_lowering=False, num_swdge_queues=4)

    d_emb = nc.dram_tensor("emb", [NCAT * VOCAB, EMB], BF16, kind="ExternalInput")
    d_catq = nc.dram_tensor("catq", [128, NTILE * NCAT], I8, kind="ExternalInput")
    d_gidx = nc.dram_tensor("gidx", [16, GIDX_COLS], I16, kind="ExternalInput")
    d_num = nc.dram_tensor("num", [128, NTILE * NNUM], F32, kind="ExternalInput")
    d_w1 = nc.dram_tensor("w1", [D, L1], BF16, kind="ExternalInput")
    d_w2 = nc.dram_tensor("w2", [L1, L2], BF16, kind="ExternalInput")
    d_w3 = nc.dram_tensor("w3", [L2, L3], BF16, kind="ExternalInput")
    d_b1 = nc.dram_tensor("b1r", [128, M1], F32, kind="ExternalInput")
    d_b2 = nc.dram_tensor("b2r", [128, M2], F32, kind="ExternalInput")
    d_b3 = nc.dram_tensor("b3r", [128, M3], F32, kind="ExternalInput")
    d_bc = nc.dram_tensor("bcr", [128, 1], F32, kind="ExternalInput")
    # avec: per k-chunk 4 columns [alpha0, alpha1, alpha2, wc_x]
    d_avec = nc.dram_tensor("avec", [128, KC * 4], BF16, kind="ExternalInput")
    d_idf = nc.dram_tensor("idf", [128, 128], F32, kind="ExternalInput")
    d_idb = nc.dram_tensor("idb", [128, 128], BF16, kind="ExternalInput")
    d_wch = nc.dram_tensor("wch", [128, 2], BF16, kind="ExternalInput")
    d_out = nc.dram_tensor("out", [128, NTILE], F32, kind="ExternalOutput")

    with tile.TileContext(nc) as tc:
        with (
            tc.tile_pool(name="consts", bufs=1) as consts,
            tc.tile_pool(name="quadp", bufs=10) as quadp,
            tc.tile_pool(name="ctp", bufs=2) as ctp,
            tc.tile_pool(name="actp", bufs=3) as actp,
            tc.tile_pool(name="rowp", bufs=1) as rowp,
            tc.tile_pool(name="ps_mm", bufs=3, space="PSUM") as ps_mm,
            tc.tile_pool(name="ps_tp", bufs=2, space="PSUM") as ps_tp,
            tc.tile_pool(name="ps_a", bufs=2, space="PSUM") as ps_a,
            tc.tile_pool(name="ps_h", bufs=1, space="PSUM") as ps_h,
        ):
            # --------- minimal-latency index upload, then gathers ---------
            # catq (cat & 3, int8, 53KB) first: masks depend on it.
            catq = consts.tile([128, NTILE * NCAT], I8, name="catq_sb")
            nc.sync.dma_start(catq[:], d_catq[:])
            # gidx replicated on-chip: phase-0 cols by sync engine,
            # the rest by the scalar engine, 8 partition-groups each.
            # gidx in two tiles so phase-0 gathers only dep on the
            # phase-0 replicas; those 8 replica DMAs dispatch from two
            # queues (sync+scalar) in parallel.
            gidx0 = consts.tile([128, _P0COLS], I16, name="gidx0_sb")
            gidx1 = consts.tile([128, GIDX_COLS - _P0COLS], I16, name="gidx1_sb")
            for g in range(8):
                eng = nc.sync if g % 2 == 0 else nc.scalar
                eng.dma_start(
                    gidx0[g * 16 : (g + 1) * 16, :], d_gidx[:, 0:_P0COLS]
                )
            # identities next on the sync queue: first PE transpose needs
            # ident_bf at ~10us
            ident_bf = consts.tile_from(d_idb[:], name="ident_bf")
            ident = consts.tile_from(d_idf[:], name="ident")
            for g in range(8):
                nc.sync.dma_start(
                    gidx1[g * 16 : (g + 1) * 16, :], d_gidx[:, _P0COLS:]
                )

            # quarter-select masks: m_i = ((cat & 3) == i), i in 1..3
            masks = []
            for i in range(1, 4):
                mi = consts.tile([128, NTILE * NCAT], I8, name=f"m{i}")
                nc.vector.tensor_single_scalar(
                    mi[:], catq[:], i, mybir.AluOpType.is_equal
                )
                masks.append(mi[:].rearrange("p (T c) -> p T c", c=NCAT))

            # combined features per (phase, k-block), batch-natural bf16
            cnb = [
                [
                    consts.tile([128, PHASES[p][1], KW[k]], BF16, name=f"cnb{p}_{k}")
                    for k in range(KC)
                ]
                for p in range(NPH)
            ]

            _g = [0]  # global gather counter for queue assignment

            def emit_gather_phase(p):
                """Per category: SWDGE gather, then quarter-select into cnb
                (scalar copy + 3 DVE predicated copies), then PE transposes
                for each completed 4-category k-block. The numerical-feature
                copies are emitted just before c=24 so the scalar queue isn't
                head-of-line blocked on the num upload early in the phase."""
                t0, nt = PHASES[p]
                nidx = nt * 128
                for c in range(NCAT):
                    if c == 24:
                        emit_numcopy(p)
                    quad = quadp.tile([128, nt, 4 * EMB], BF16, name=f"quad{nt}",
                                      tag=f"quad{nt}")
                    nc.gpsimd.dma_gather(
                        out_ap=quad[:],
                        in_ap=d_emb[c * VOCAB : (c + 1) * VOCAB, :].rearrange(
                            "(r q) e -> r (q e)", q=4
                        ),
                        idxs_ap=(gidx0 if p == 0 else gidx1)[
                            :,
                            _PHOFF[p] - (0 if p == 0 else _P0COLS)
                            + c * nt * 8 : _PHOFF[p]
                            - (0 if p == 0 else _P0COLS)
                            + (c + 1) * nt * 8,
                        ],
                        num_idxs=nidx,
                        num_idxs_reg=nidx,
                        elem_size=4 * EMB,
                        queue_num=(
                            queue_map[_g[0]] if queue_map else _g[0] % 4
                        ),
                    )
                    _g[0] += 1
                    co = (c % CPB) * EMB
                    dest = cnb[p][c // CPB][:, :, co : co + EMB]
                    nc.scalar.copy(dest, quad[:, :, 0:EMB])
                    for i in range(1, 4):
                        mslice = masks[i - 1][:, t0 : t0 + nt, c : c + 1]
                        mb, _ = broadcast_tensor_aps(mslice, dest)
                        nc.vector.copy_predicated(
                            dest, mb, quad[:, :, i * EMB : (i + 1) * EMB]
                        )
                    if c % CPB == CPB - 1 and c // CPB < 6:
                        emit_transposes(p, c // CPB)
                    if c == NCAT - 1:
                        emit_transposes(p, 6)

            ct_tiles = {}
            psa_tiles = {}
            psh_tiles = {}

            def emit_transposes(p, k):
                # k<6 (kw=128): DMA xbar transpose SBUF->SBUF on the sync
                # (SP) HWDGE queue -- keeps the 16x7 per-core transposes
                # and their PSUM drains off the PE/DVE critical path.
                # k=6 (kw=77 < xbar 128-col tile): PE transpose as before.
                kw = KW[k]
                pt0, pnt = PHASES[p]
                for ci, (t0, nt, cp) in enumerate(CHUNKS):
                    if cp != p:
                        continue
                    ctk = ctp.tile([128, 512], BF16, name=f"ct{k}", tag=f"ct{k}")
                    ct_tiles[(ci, k)] = ctk
                    for t in range(nt):
                        tt = (t0 - pt0) + t
                        if k < 6:
                            nc.sync.dma_start_transpose(
                                ctk[:, t * 128 : (t + 1) * 128],
                                cnb[p][k][:, tt, :],
                            )
                        else:
                            pst = ps_tp.tile([128, 128], BF16, name="pst", tag="pst")
                            nc.tensor.transpose(
                                pst[0:kw, :],
                                cnb[p][k][:, tt, 0:kw],
                                ident_bf[:],
                            )
                            nc.any.tensor_copy(
                                ctk[0:kw, t * 128 : (t + 1) * 128], pst[0:kw, :]
                            )

            def emit_chunk(ci):
                t0, nt, cp = CHUNKS[ci]
                w = nt * 128
                ct = [ct_tiles[(ci, k)] for k in range(KC)]
                # cross-net dot products: [alpha0, alpha1, alpha2, wc_x]
                psa = ps_a.tile([4, 512], F32, name="psa", tag="psa")
                for k in range(KC):
                    kw = KW[k]
                    nc.tensor.matmul(
                        psa[:, 0:w],
                        avec[0:kw, k * 4 : (k + 1) * 4],
                        ct[k][0:kw, 0:w],
                        start=(k == 0),
                        stop=(k == KC - 1),
                    )
                # MLP
                h1 = []
                for m in range(M1):
                    psm = ps_mm.tile([128, 512], F32, name="psm")
                    for k in range(KC):
                        kw = KW[k]
                        nc.tensor.matmul(
                            psm[:, 0:w],
                            w1[k][0:kw, m * 128 : (m + 1) * 128],
                            ct[k][0:kw, 0:w],
                            start=(k == 0),
                            stop=(k == KC - 1),
                        )
                    h = actp.tile([128, 512], BF16, name=f"h1_{m}", tag=f"h1_{m}")
                    nc.scalar.activation(
                        h[:, 0:w], psm[:, 0:w], mybir.ActivationFunctionType.Relu,
                        bias=b1r[:, m : m + 1],
                    )
                    h1.append(h)
                h2 = []
                for m in range(M2):
                    psm = ps_mm.tile([128, 512], F32, name="psm")
                    for k in range(M1):
                        nc.tensor.matmul(
                            psm[:, 0:w],
                            w2[k][:, m * 128 : (m + 1) * 128],
                            h1[k][:, 0:w],
                            start=(k == 0),
                            stop=(k == M1 - 1),
                        )
                    h = actp.tile([128, 512], BF16, name=f"h2_{m}", tag=f"h2_{m}")
                    nc.scalar.activation(
                        h[:, 0:w], psm[:, 0:w], mybir.ActivationFunctionType.Relu,
                        bias=b2r[:, m : m + 1],
                    )
                    h2.append(h)
                h3 = []
                for m in range(M3):
                    psm = ps_mm.tile([128, 512], F32, name="psm")
                    for k in range(M2):
                        nc.tensor.matmul(
                            psm[:, 0:w],
                            w3[k][:, m * 128 : (m + 1) * 128],
                            h2[k][:, 0:w],
                            start=(k == 0),
                            stop=(k == M2 - 1),
                        )
                    h = actp.tile([128, 512], BF16, name=f"h3_{m}", tag=f"h3_{m}")
                    nc.scalar.activation(
                        h[:, 0:w], psm[:, 0:w],
                        mybir.ActivationFunctionType.Identity,
                        bias=b3r[:, m : m + 1],
                    )
                    h3.append(h)

                # h3 . wc_h -> row
                psh = ps_h.tile([1, 512], F32, name="psh", tag="psrow")
                for j in range(M3):
                    nc.tensor.matmul(
                        psh[:, 0:w], wch[:, j : j + 1], h3[j][:, 0:w],
                        start=(j == 0), stop=(j == M3 - 1),
                    )
                psa_tiles[ci] = psa
                psh_tiles[ci] = psh

            # ------------- final combine (batch-natural, per chunk) --------
            # x3 = p3*x0 + q30*b0 + q31*b1 + b2 with per-sample scalars from
            # the a-dots; Wc_x.x3 folds to p3*awc + q30*d0 + q31*d1 + d2.
            def emit_fin(ci):
                t0, nt, cp = CHUNKS[ci]
                w = nt * 128
                a_sb = actp.tile([4, 512], F32, name="a_sb", tag="a_sb")
                nc.any.tensor_copy(a_sb[:, 0:w], psa_tiles[ci][:, 0:w])
                h_sb = actp.tile([1, 512], F32, name="h_sb", tag="h_sb")
                nc.any.tensor_copy(h_sb[:, 0:w], psh_tiles[ci][:, 0:w])
                for t in range(nt):
                    pta = ps_tp.tile([128, 4], F32, name="pta", tag="pst")
                    nc.tensor.transpose(
                        pta[:], a_sb[:, t * 128 : (t + 1) * 128], ident[0:4, 0:4]
                    )
                    T = t0 + t
                    nc.vector.tensor_copy(a_nat[:, T * 4 : (T + 1) * 4], pta[:])
                    pth = ps_tp.tile([128, 1], F32, name="pth", tag="pst")
                    nc.tensor.transpose(
                        pth[:], h_sb[:, t * 128 : (t + 1) * 128], ident[0:1, 0:1]
                    )
                    nc.vector.tensor_copy(h_nat[:, T : T + 1], pth[:])
                av = a_nat[:, t0 * 4 : (t0 + nt) * 4].rearrange(
                    "p (t l) -> p t l", l=4
                )
                a0, a1, a2, awc = (av[:, :, l] for l in range(4))
                hn = h_nat[:, t0 : t0 + nt]

                def rtile(name):
                    return rowp.tile([128, nt], F32, name=name, tag=f"{name}_{ci}")

                p1 = rtile("p1")            # 1 + s0
                nc.vector.tensor_scalar_add(p1[:], a0, 1.0)
                s1 = rtile("s1")            # s1 = p1*a1 (+ c10)
                nc.vector.tensor_mul(s1[:], a1, p1[:])
                if c10 != 0.0:
                    nc.vector.tensor_scalar_add(s1[:], s1[:], float(c10))
                u1 = rtile("u1")            # 1 + s1  (= q20)
                nc.vector.tensor_scalar_add(u1[:], s1[:], 1.0)
                p2 = rtile("p2")
                nc.vector.tensor_mul(p2[:], p1[:], u1[:])
                s2 = rtile("s2")            # s2 = p2*a2 + u1*c20 + c21
                nc.vector.tensor_mul(s2[:], a2, p2[:])
                if c20 != 0.0:
                    v20 = rtile("v20")
                    nc.vector.tensor_scalar_mul(v20[:], u1[:], float(c20))
                    nc.vector.tensor_add(s2[:], s2[:], v20[:])
                if c21 != 0.0:
                    nc.vector.tensor_scalar_add(s2[:], s2[:], float(c21))
                u2 = rtile("u2")            # 1 + s2
                nc.vector.tensor_scalar_add(u2[:], s2[:], 1.0)
                p3 = rtile("p3")
                nc.vector.tensor_mul(p3[:], p2[:], u2[:])
                fin = rtile("fin")          # awc*p3 (+ bias-derived terms)
                nc.vector.tensor_mul(fin[:], awc, p3[:])
                if d0 != 0.0:
                    q30 = rtile("q30")
                    nc.vector.tensor_mul(q30[:], u1[:], u2[:])
                    nc.vector.tensor_scalar_mul(q30[:], q30[:], float(d0))
                    nc.vector.tensor_add(fin[:], fin[:], q30[:])
                if d1 != 0.0:
                    w1t = rtile("w1t")
                    nc.vector.tensor_scalar_mul(w1t[:], u2[:], float(d1))
                    nc.vector.tensor_add(fin[:], fin[:], w1t[:])
                if d2 != 0.0:
                    nc.vector.tensor_scalar_add(fin[:], fin[:], float(d2))
                nc.vector.tensor_add(fin[:], fin[:], hn)
                ons = out_nat[:, t0 : t0 + nt]
                nc.scalar.activation(
                    ons, fin[:], mybir.ActivationFunctionType.Sigmoid,
                    bias=bcr[:, 0:1],
                )
                nc.sync.dma_start(d_out[:, t0 : t0 + nt], ons)

            # constants / weights (upload overlaps the gather stream)
            num_sb = consts.tile([128, NTILE * NNUM], F32, name="num_sb")
            nc.sync.dma_start(num_sb[:], d_num[:])
            avec = consts.tile_from(d_avec[:], name="avec_sb")
            wch = consts.tile_from(d_wch[:], name="wch_sb")
            w1 = [
                consts.tile_from(d_w1[k * 128 : k * 128 + KW[k], :], name=f"w1_{k}")
                for k in range(KC)
            ]
            w2 = [
                consts.tile_from(d_w2[k * 128 : (k + 1) * 128, :], name=f"w2_{k}")
                for k in range(M1)
            ]
            w3 = [
                consts.tile_from(d_w3[k * 128 : (k + 1) * 128, :], name=f"w3_{k}")
                for k in range(M2)
            ]
            b1r = consts.tile_from(d_b1[:], name="b1r_sb")
            b2r = consts.tile_from(d_b2[:], name="b2r_sb")
            b3r = consts.tile_from(d_b3[:], name="b3r_sb")
            bcr = consts.tile_from(d_bc[:], name="bcr_sb")

            warm = ps_tp.tile([128, 4], F32, name="warm", tag="pst")
            nc.tensor.transpose(warm[0:4, 0:4], ident[0:4, 0:4], ident[0:4, 0:4])

            # natural-layout accumulators for the final combine
            a_nat = consts.tile([128, NTILE * 4], F32, name="a_nat")
            h_nat = consts.tile([128, NTILE], F32, name="h_nat")
            out_nat = consts.tile([128, NTILE], F32, name="out_nat")

            # numerical features (block 6 cols 64:77), per phase
            def emit_numcopy(p):
                t0, nt = PHASES[p]
                for tt in range(nt):
                    T = t0 + tt
                    nc.scalar.copy(
                        cnb[p][6][:, tt, 2 * EMB : KW[6]],
                        num_sb[:, T * NNUM : (T + 1) * NNUM],
                    )

            # ---------------- emission schedule ----------------
            # PE program order: p0 transposes, c0, c1, p1 tp, c2, p2 tp,
            # fin0 tp, c3, p3 tp, fin1 tp, c4, fin2-4 -- each chunk's
            # matmuls sit before the NEXT phase's transposes so compute
            # never queues behind not-yet-gathered data.
            emit_gather_phase(0)
            emit_chunk(0)
            emit_chunk(1)
            emit_gather_phase(1)
            emit_chunk(2)
            emit_gather_phase(2)
            emit_fin(0)
            emit_chunk(3)
            emit_gather_phase(3)
            emit_fin(1)
            emit_chunk(4)
            emit_fin(2)
            emit_fin(3)
            emit_fin(4)

    nc.compile()
    return nc


_CACHE: dict = {}


def _gather_lanes(nc) -> list:
    """Per-gather DMASW lane (emission order) from the tile sem assigner."""
    import re

    gath = []
    for blk in nc.m.functions[0].blocks:
        for inst in blk.instructions:
            if type(inst).__name__ == "InstDMAGatherAnt":
                lane = None
                for u in inst.sync_info.on_update or []:
                    m = re.match(r"DMASW(\d+)_", u.ant_name or "")
                    if m:
                        lane = int(m.group(1))
                gath.append((int(inst.name.split("-")[1]), lane))
    gath.sort()
    return [lane for _, lane in gath]


def _get_nc(cross_consts) -> bass.Bass:
    """Two-pass build: the tile scheduler assigns SWDGE completion sems
    to the 8 DMASW lanes round-robin in ITS instruction order, which can
    diverge from emission order.  Each physical sem is queue-locked, so the
    gather's SWDGE queue must equal its assigned lane % 4.  Pass 1 builds
    with a nominal rotation to read the lane assignment; pass 2 rebuilds
    with queue_num = lane % 4 (queue_num doesn't affect scheduling, so the
    assignment is identical across passes)."""
    key = cross_consts
    if key not in _CACHE:
        probe = _build(cross_consts)
        qmap = [lane % 4 for lane in _gather_lanes(probe)]
        _CACHE[key] = _build(cross_consts, queue_map=qmap)
    return _CACHE[key]


def kernel(
    categorical_input,
    numerical_input,
    emb_tables,
    alphas,
    cross_bias,
    W1, b1, W2, b2, W3, b3, Wc, bc,
) -> np.ndarray:
    cat = np.ascontiguousarray(np.asarray(categorical_input, dtype=np.int64))
    num = np.ascontiguousarray(np.asarray(numerical_input, dtype=np.float32))
    emb = np.ascontiguousarray(
        np.asarray(emb_tables, dtype=np.float32).reshape(NCAT * VOCAB, EMB)
    )
    alphas = np.asarray(alphas, dtype=np.float32)
    cross_bias = np.asarray(cross_bias, dtype=np.float32)
    W1 = np.ascontiguousarray(np.asarray(W1, dtype=np.float32))
    W2 = np.ascontiguousarray(np.asarray(W2, dtype=np.float32))
    W3 = np.ascontiguousarray(np.asarray(W3, dtype=np.float32))
    Wc = np.asarray(Wc, dtype=np.float32)
    b1 = np.asarray(b1, dtype=np.float32)
    b2 = np.asarray(b2, dtype=np.float32)
    b3 = np.asarray(b3, dtype=np.float32)
    bc = np.asarray(bc, dtype=np.float32)

    # host scalar constants folding cross_bias into the per-sample chain
    cross_consts = (
        float(np.dot(alphas[1], cross_bias[0])),
        float(np.dot(alphas[2], cross_bias[0])),
        float(np.dot(alphas[2], cross_bias[1])),
        float(np.dot(Wc[:D, 0], cross_bias[0])),
        float(np.dot(Wc[:D, 0], cross_bias[1])),
        float(np.dot(Wc[:D, 0], cross_bias[2])),
    )
    nc = _get_nc(cross_consts)

    def to_dev(v):  # [D(,k)] -> [KC*128(,k)] zero-padded
        shape = (KC * 128,) + v.shape[1:]
        p = np.zeros(shape, np.float32)
        p[:D] = v
        return p

    def pad_col(v):  # [845] -> [128, KC] column-chunked, zero-padded
        return to_dev(v).reshape(KC, 128).T.copy()

    avec = np.zeros((128, KC * 4), np.float32)
    for l in range(NCROSS):
        avec[:, l::4] = pad_col(alphas[l])
    avec[:, 3::4] = pad_col(Wc[:D, 0])
    wch = Wc[D : D + L3, 0].reshape(2, 128).T.copy()
    b1r = b1.reshape(M1, 128).T.copy()
    b2r = b2.reshape(M2, 128).T.copy()
    b3r = b3.reshape(M3, 128).T.copy()
    bcr = np.broadcast_to(bc.reshape(1, 1), (128, 1)).copy()

    import ml_dtypes

    bf = ml_dtypes.bfloat16
    common = {
        "emb": emb.astype(bf),
        "w1": W1.astype(bf),
        "w2": W2.astype(bf),
        "w3": W3.astype(bf),
        "b1r": b1r,
        "b2r": b2r,
        "b3r": b3r,
        "bcr": bcr,
        "avec": avec.astype(bf),
        "wch": wch.astype(bf),
        "idf": np.eye(128, dtype=np.float32),
        "idb": np.eye(128, dtype=np.float32).astype(bf),
    }
    in_maps = []
    for core in range(NCORES):
        cs = cat[core * BC : (core + 1) * BC].astype(np.int32)  # [2048, 26]
        ns = num[core * BC : (core + 1) * BC]
        catq = np.ascontiguousarray(
            (cs & 3)
            .astype(np.int8)
            .reshape(NTILE, 128, NCAT)
            .transpose(1, 0, 2)
            .reshape(128, NTILE * NCAT)
        )
        numr = np.ascontiguousarray(
            ns.reshape(NTILE, 128, NNUM).transpose(1, 0, 2).reshape(128, NTILE * NNUM)
        )
        # gather indices: per (phase, category) block, int16 v//4,
        # lookup i at [i % 16, i // 16]; single 16-row copy (the kernel
        # replicates to the 8 partition groups on-chip)
        gi = np.zeros((16, GIDX_COLS), np.int16)
        for p, (t0, nt) in enumerate(PHASES):
            nb = nt * 128
            vs = cs[t0 * 128 : t0 * 128 + nb]  # [nb, 26]
            q4 = (vs // 4).astype(np.int16)
            wrapped = q4.reshape(nb // 16, 16, NCAT).transpose(1, 0, 2)
            for c in range(NCAT):
                blk = _PHOFF[p] + c * nt * 8
                gi[:, blk : blk + nt * 8] = wrapped[:, :, c]
        in_maps.append({**common, "catq": catq, "num": numr, "gidx": gi})

    res = run_bass_kernel_spmd(nc, in_maps, core_ids=list(range(NCORES)))
    outs = []
    for core in range(NCORES):
        o = res.results[core]["out"]  # [128, NTILE], sample T*128+p at [p, T]
        outs.append(o.T.reshape(BC, 1))
    return np.concatenate(outs, axis=0).astype(np.float32)


# revision 12
# speedup vs baseline: 1.9184x; 1.9184x over previous
"""DCN (DLRM-style deep & cross network) Trainium2 Bass kernel.

Sharding: data-parallel over batch across 8 NeuronCores (2048 samples/core).
Embedding tables (bf16) + MLP weights are replicated to every core's HBM.

Per-core pipeline (activations kept feature-major, i.e. transposed, for PE):
  1. Embedding gather via InstDMAGatherAnt: one instruction per
     (phase, category), fetching 256B quad-rows (4 bf16 vocab rows, the
     minimum 256B element) with int16 indices v//4, spread across all 4
     SWDGE queues.  Measured floor is ~2.5ns/descriptor (marginal) +
     ~1.15us fixed per instruction, so phases use the largest legal
     nidx (1024) up front and a small 256 tail: [1024, 768, 256].
  2. gidx is uploaded as a single [16, cols] tensor and replicated to
     the 8 16-partition groups by on-chip DMAs so the first gather
     launches within a few us (no 850KB serial upload in front).
  3. scalar copy + 3x copy_predicated (DVE) select the right 32-value
     quarter per lookup into the combined feature tile (no DMA-engine
     traffic besides the gathers themselves).
  4. PE 128x128 transposes -> ct[k] = combined^T chunks [128, <=512].
  5. MLP: h^T = relu(W^T @ x^T) chains, bf16 matmuls, fp32 accumulate.
  6. CrossNet folds to 4 packed dot products against x0 (alpha0..2,
     Wc_x) plus a scalar chain with host-precomputed alpha_l.b_j
     constants; final = sigmoid(x-part + Wc_h . h3 + bc).
"""

import numpy as np

import concourse.bass as bass
import concourse.mybir as mybir
import concourse.tile as tile
from concourse import bacc
from concourse.bass import broadcast_tensor_aps
from concourse.bass_utils import run_bass_kernel_spmd
F32 = mybir.dt.float32
BF16 = mybir.dt.bfloat16
I32 = mybir.dt.int32
I16 = mybir.dt.int16
I8 = mybir.dt.int8

B = 16384
NCORES = 8
BC = B // NCORES            # 2048 samples per core
NCAT = 26
VOCAB = 100000
EMB = 32
NNUM = 13
D = NCAT * EMB + NNUM       # 845

L1, L2, L3 = 1024, 512, 256
NCROSS = 3
KC = 7                      # feature chunks of 128 (6*128 + 77)
KW = [128] * 6 + [D - 6 * 128]
CPB = 128 // EMB            # 4 categories per 128-feature block
M1, M2, M3 = L1 // 128, L2 // 128, L3 // 128   # 8, 4, 2
NTILE = BC // 128           # 16 batch tiles per core

# gather phases: (first tile, #tiles); nidx = #tiles * 128 (max 1024).
# Front-loaded 1024 phase amortizes the ~1.15us/instruction fixed cost;
# the small late phases keep the post-gather compute tail short.
PHASES = [(0, 4), (4, 4), (8, 4), (12, 2), (14, 2)]
NPH = len(PHASES)
# compute chunks: (first tile, #tiles, phase fed by)
CHUNKS = [(0, 4, 0), (4, 4, 1), (8, 4, 2), (12, 2, 3), (14, 2, 4)]
NCHUNK = len(CHUNKS)
# idx column offset (in 16-partition-wrapped cols) per phase block
_PHOFF = [0]
for _t0, _nt in PHASES:
    _PHOFF.append(_PHOFF[-1] + NCAT * _nt * 8)
GIDX_COLS = _PHOFF[-1]      # 3328 total
_P0COLS = _PHOFF[1]         # phase-0 block


def _build(cross_consts, queue_map=None) -> bass.Bass:
    # cross_consts = (c10, c20, c21, d0, d1, d2):
    #   c_lj = alpha_l . cross_bias_j,  d_j = Wc_x . cross_bias_j
    c10, c20, c21, d0, d1, d2 = cross_consts

    nc = bacc.Bacc("TRN2", target_bir_lowering=False, num_swdge_queues=4)

    d_emb = nc.dram_tensor("emb", [NCAT * VOCAB, EMB], BF16, kind="ExternalInput")
    d_catq = nc.dram_tensor("catq", [128, NTILE * NCAT], I8, kind="ExternalInput")
    d_gidx = nc.dram_tensor("gidx", [16, GIDX_COLS], I16, kind="ExternalInput")
    d_num = nc.dram_tensor("num", [128, NTILE * NNUM], F32, kind="ExternalInput")
    d_w1 = nc.dram_tensor("w1", [D, L1], BF16, kind="ExternalInput")
    d_w2 = nc.dram_tensor("w2", [L1, L2], BF16, kind="ExternalInput")
    d_w3 = nc.dram_tensor("w3", [L2, L3], BF16, kind="ExternalInput")
    d_b1 = nc.dram_tensor("b1r", [128, M1], F32, kind="ExternalInput")
    d_b2 = nc.dram_tensor("b2r", [128, M2], F32, kind="ExternalInput")
    d_b3 = nc.dram_tensor("b3r", [128, M3], F32, kind="ExternalInput")
    d_bc = nc.dram_tensor("bcr", [128, 1], F32, kind="ExternalInput")
    # avec: per k-chunk 4 columns [alpha0, alpha1, alpha2, wc_x]
    d_avec = nc.dram_tensor("avec", [128, KC * 4], BF16, kind="ExternalInput")
    d_idf = nc.dram_tensor("idf", [128, 128], F32, kind="ExternalInput")
    d_idb = nc.dram_tensor("idb", [128, 128], BF16, kind="ExternalInput")
    d_wch = nc.dram_tensor("wch", [128, 2], BF16, kind="ExternalInput")
    d_out = nc.dram_tensor("out", [128, NTILE], F32, kind="ExternalOutput")

    with tile.TileContext(nc) as tc:
        with (
            tc.tile_pool(name="consts", bufs=1) as consts,
            tc.tile_pool(name="quadp", bufs=10) as quadp,
            tc.tile_pool(name="ctp", bufs=2) as ctp,
            tc.tile_pool(name="actp", bufs=3) as actp,
            tc.tile_pool(name="rowp", bufs=1) as rowp,
            tc.tile_pool(name="ps_mm", bufs=3, space="PSUM") as ps_mm,
            tc.tile_pool(name="ps_tp", bufs=2, space="PSUM") as ps_tp,
            tc.tile_pool(name="ps_a", bufs=2, space="PSUM") as ps_a,
            tc.tile_pool(name="ps_h", bufs=1, space="PSUM") as ps_h,
        ):
            # --------- minimal-latency index upload, then gathers ---------
            # catq (cat & 3, int8, 53KB) first: masks depend on it.
            catq = consts.tile([128, NTILE * NCAT], I8, name="catq_sb")
            nc.sync.dma_start(catq[:], d_catq[:])
            # gidx replicated on-chip: phase-0 cols by sync engine,
            # the rest by the scalar engine, 8 partition-groups each.
            # gidx in two tiles so phase-0 gathers only dep on the
            # phase-0 replicas; those 8 replica DMAs dispatch from two
            # queues (sync+scalar) in parallel.
            gidx0 = consts.tile([128, _P0COLS], I16, name="gidx0_sb")
            gidx1 = consts.tile([128, GIDX_COLS - _P0COLS], I16, name="gidx1_sb")
            for g in range(8):
                eng = nc.sync if g % 2 == 0 else nc.scalar
                eng.dma_start(
                    gidx0[g * 16 : (g + 1) * 16, :], d_gidx[:, 0:_P0COLS]
                )
            # identities next on the sync queue: first PE transpose needs
            # ident_bf at ~10us
            ident_bf = consts.tile_from(d_idb[:], name="ident_bf")
            ident = consts.tile_from(d_idf[:], name="ident")
            for g in range(8):
                nc.sync.dma_start(
                    gidx1[g * 16 : (g + 1) * 16, :], d_gidx[:, _P0COLS:]
                )

            # quarter-select masks: m_i = ((cat & 3) == i), i in 1..3
            masks = []
            for i in range(1, 4):
                mi = consts.tile([128, NTILE * NCAT], I8, name=f"m{i}")
                nc.vector.tensor_single_scalar(
                    mi[:], catq[:], i, mybir.AluOpType.is_equal
                )
                masks.append(mi[:].rearrange("p (T c) -> p T c", c=NCAT))

            # combined features per (phase, k-block), batch-natural bf16
            cnb = [
                [
                    consts.tile([128, PHASES[p][1], KW[k]], BF16, name=f"cnb{p}_{k}")
                    for k in range(KC)
                ]
                for p in range(NPH)
            ]

            _g = [0]  # global gather counter for queue assignment

            def emit_gather_phase(p):
                """Per category: SWDGE gather, then quarter-select into cnb
                (scalar copy + 3 DVE predicated copies), then PE transposes
                for each completed 4-category k-block. The numerical-feature
                copies are emitted just before c=24 so the scalar queue isn't
                head-of-line blocked on the num upload early in the phase."""
                t0, nt = PHASES[p]
                nidx = nt * 128
                for c in range(NCAT):
                    if c == 24:
                        emit_numcopy(p)
                    quad = quadp.tile([128, nt, 4 * EMB], BF16, name=f"quad{nt}",
                                      tag=f"quad{nt}")
                    nc.gpsimd.dma_gather(
                        out_ap=quad[:],
                        in_ap=d_emb[c * VOCAB : (c + 1) * VOCAB, :].rearrange(
                            "(r q) e -> r (q e)", q=4
                        ),
                        idxs_ap=(gidx0 if p == 0 else gidx1)[
                            :,
                            _PHOFF[p] - (0 if p == 0 else _P0COLS)
                            + c * nt * 8 : _PHOFF[p]
                            - (0 if p == 0 else _P0COLS)
                            + (c + 1) * nt * 8,
                        ],
                        num_idxs=nidx,
                        num_idxs_reg=nidx,
                        elem_size=4 * EMB,
                        queue_num=(
                            queue_map[_g[0]] if queue_map else _g[0] % 4
                        ),
                    )
                    _g[0] += 1
                    co = (c % CPB) * EMB
                    dest = cnb[p][c // CPB][:, :, co : co + EMB]
                    nc.scalar.copy(dest, quad[:, :, 0:EMB])
                    for i in range(1, 4):
                        mslice = masks[i - 1][:, t0 : t0 + nt, c : c + 1]
                        mb, _ = broadcast_tensor_aps(mslice, dest)
                        nc.vector.copy_predicated(
                            dest, mb, quad[:, :, i * EMB : (i + 1) * EMB]
                        )
                    if c % CPB == CPB - 1 and c // CPB < 6:
                        emit_transposes(p, c // CPB)
                    if c == NCAT - 1:
                        emit_transposes(p, 6)

            ct_tiles = {}
            psa_tiles = {}
            psh_tiles = {}

            def emit_transposes(p, k):
                kw = KW[k]
                pt0, pnt = PHASES[p]
                for ci, (t0, nt, cp) in enumerate(CHUNKS):
                    if cp != p:
                        continue
                    ctk = ctp.tile([128, 512], BF16, name=f"ct{k}", tag=f"ct{k}")
                    ct_tiles[(ci, k)] = ctk
                    for t in range(nt):
                        tt = (t0 - pt0) + t
                        pst = ps_tp.tile([128, 128], BF16, name="pst", tag="pst")
                        nc.tensor.transpose(
                            pst[0:kw, :],
                            cnb[p][k][:, tt, 0:kw],
                            ident_bf[:],
                        )
                        nc.any.tensor_copy(
                            ctk[0:kw, t * 128 : (t + 1) * 128], pst[0:kw, :]
                        )

            def emit_chunk(ci):
                t0, nt, cp = CHUNKS[ci]
                w = nt * 128
                ct = [ct_tiles[(ci, k)] for k in range(KC)]
                # cross-net dot products: [alpha0, alpha1, alpha2, wc_x]
                psa = ps_a.tile([4, 512], F32, name="psa", tag="psa")
                for k in range(KC):
                    kw = KW[k]
                    nc.tensor.matmul(
                        psa[:, 0:w],
                        avec[0:kw, k * 4 : (k + 1) * 4],
                        ct[k][0:kw, 0:w],
                        start=(k == 0),
                        stop=(k == KC - 1),
                    )
                # MLP
                h1 = []
                for m in range(M1):
                    psm = ps_mm.tile([128, 512], F32, name="psm")
                    for k in range(KC):
                        kw = KW[k]
                        nc.tensor.matmul(
                            psm[:, 0:w],
                            w1[k][0:kw, m * 128 : (m + 1) * 128],
                            ct[k][0:kw, 0:w],
                            start=(k == 0),
                            stop=(k == KC - 1),
                        )
                    h = actp.tile([128, 512], BF16, name=f"h1_{m}", tag=f"h1_{m}")
                    nc.scalar.activation(
                        h[:, 0:w], psm[:, 0:w], mybir.ActivationFunctionType.Relu,
                        bias=b1r[:, m : m + 1],
                    )
                    h1.append(h)
                h2 = []
                for m in range(M2):
                    psm = ps_mm.tile([128, 512], F32, name="psm")
                    for k in range(M1):
                        nc.tensor.matmul(
                            psm[:, 0:w],
                            w2[k][:, m * 128 : (m + 1) * 128],
                            h1[k][:, 0:w],
                            start=(k == 0),
                            stop=(k == M1 - 1),
                        )
                    h = actp.tile([128, 512], BF16, name=f"h2_{m}", tag=f"h2_{m}")
                    nc.scalar.activation(
                        h[:, 0:w], psm[:, 0:w], mybir.ActivationFunctionType.Relu,
                        bias=b2r[:, m : m + 1],
                    )
                    h2.append(h)
                h3 = []
                for m in range(M3):
                    psm = ps_mm.tile([128, 512], F32, name="psm")
                    for k in range(M2):
                        nc.tensor.matmul(
                            psm[:, 0:w],
                            w3[k][:, m * 128 : (m + 1) * 128],
                            h2[k][:, 0:w],
                            start=(k == 0),
                            stop=(k == M2 - 1),
                        )
                    h = actp.tile([128, 512], BF16, name=f"h3_{m}", tag=f"h3_{m}")
                    nc.scalar.activation(
                        h[:, 0:w], psm[:, 0:w],
                        mybir.ActivationFunctionType.Identity,
                        bias=b3r[:, m : m + 1],
                    )
                    h3.append(h)

                # h3 . wc_h -> row
                psh = ps_h.tile([1, 512], F32, name="psh", tag="psrow")
                for j in range(M3):
                    nc.tensor.matmul(
                        psh[:, 0:w], wch[:, j : j + 1], h3[j][:, 0:w],
                        start=(j == 0), stop=(j == M3 - 1),
                    )
                psa_tiles[ci] = psa
                psh_tiles[ci] = psh

            # ------------- final combine (batch-natural, per chunk) --------
            # x3 = p3*x0 + q30*b0 + q31*b1 + b2 with per-sample scalars from
            # the a-dots; Wc_x.x3 folds to p3*awc + q30*d0 + q31*d1 + d2.
            def emit_fin(ci):
                t0, nt, cp = CHUNKS[ci]
                w = nt * 128
                a_sb = actp.tile([4, 512], F32, name="a_sb", tag="a_sb")
                nc.any.tensor_copy(a_sb[:, 0:w], psa_tiles[ci][:, 0:w])
                h_sb = actp.tile([1, 512], F32, name="h_sb", tag="h_sb")
                nc.any.tensor_copy(h_sb[:, 0:w], psh_tiles[ci][:, 0:w])
                for t in range(nt):
                    pta = ps_tp.tile([128, 4], F32, name="pta", tag="pst")
                    nc.tensor.transpose(
                        pta[:], a_sb[:, t * 128 : (t + 1) * 128], ident[0:4, 0:4]
                    )
                    T = t0 + t
                    nc.vector.tensor_copy(a_nat[:, T * 4 : (T + 1) * 4], pta[:])
                    pth = ps_tp.tile([128, 1], F32, name="pth", tag="pst")
                    nc.tensor.transpose(
                        pth[:], h_sb[:, t * 128 : (t + 1) * 128], ident[0:1, 0:1]
                    )
                    nc.vector.tensor_copy(h_nat[:, T : T + 1], pth[:])
                av = a_nat[:, t0 * 4 : (t0 + nt) * 4].rearrange(
                    "p (t l) -> p t l", l=4
                )
                a0, a1, a2, awc = (av[:, :, l] for l in range(4))
                hn = h_nat[:, t0 : t0 + nt]

                def rtile(name):
                    return rowp.tile([128, nt], F32, name=name, tag=f"{name}_{ci}")

                p1 = rtile("p1")            # 1 + s0
                nc.vector.tensor_scalar_add(p1[:], a0, 1.0)
                s1 = rtile("s1")            # s1 = p1*a1 (+ c10)
                nc.vector.tensor_mul(s1[:], a1, p1[:])
                if c10 != 0.0:
                    nc.vector.tensor_scalar_add(s1[:], s1[:], float(c10))
                u1 = rtile("u1")            # 1 + s1  (= q20)
                nc.vector.tensor_scalar_add(u1[:], s1[:], 1.0)
                p2 = rtile("p2")
                nc.vector.tensor_mul(p2[:], p1[:], u1[:])
                s2 = rtile("s2")            # s2 = p2*a2 + u1*c20 + c21
                nc.vector.tensor_mul(s2[:], a2, p2[:])
                if c20 != 0.0:
                    v20 = rtile("v20")
                    nc.vector.tensor_scalar_mul(v20[:], u1[:], float(c20))
                    nc.vector.tensor_add(s2[:], s2[:], v20[:])
                if c21 != 0.0:
                    nc.vector.tensor_scalar_add(s2[:], s2[:], float(c21))
                u2 = rtile("u2")            # 1 + s2
                nc.vector.tensor_scalar_add(u2[:], s2[:], 1.0)
                p3 = rtile("p3")
                nc.vector.tensor_mul(p3[:], p2[:], u2[:])
                fin = rtile("fin")          # awc*p3 (+ bias-derived terms)
                nc.vector.tensor_mul(fin[:], awc, p3[:])
                if d0 != 0.0:
                    q30 = rtile("q30")
                    nc.vector.tensor_mul(q30[:], u1[:], u2[:])
                    nc.vector.tensor_scalar_mul(q30[:], q30[:], float(d0))
                    nc.vector.tensor_add(fin[:], fin[:], q30[:])
                if d1 != 0.0:
                    w1t = rtile("w1t")
                    nc.vector.tensor_scalar_mul(w1t[:], u2[:], float(d1))
                    nc.vector.tensor_add(fin[:], fin[:], w1t[:])
                if d2 != 0.0:
                    nc.vector.tensor_scalar_add(fin[:], fin[:], float(d2))
                nc.vector.tensor_add(fin[:], fin[:], hn)
                ons = out_nat[:, t0 : t0 + nt]
                nc.scalar.activation(
                    ons, fin[:], mybir.ActivationFunctionType.Sigmoid,
                    bias=bcr[:, 0:1],
                )
                nc.sync.dma_start(d_out[:, t0 : t0 + nt], ons)

            # constants / weights (upload overlaps the gather stream)
            num_sb = consts.tile([128, NTILE * NNUM], F32, name="num_sb")
            nc.sync.dma_start(num_sb[:], d_num[:])
            avec = consts.tile_from(d_avec[:], name="avec_sb")
            wch = consts.tile_from(d_wch[:], name="wch_sb")
            w1 = [
                consts.tile_from(d_w1[k * 128 : k * 128 + KW[k], :], name=f"w1_{k}")
                for k in range(KC)
            ]
            w2 = [
                consts.tile_from(d_w2[k * 128 : (k + 1) * 128, :], name=f"w2_{k}")
                for k in range(M1)
            ]
            w3 = [
                consts.tile_from(d_w3[k * 128 : (k + 1) * 128, :], name=f"w3_{k}")
                for k in range(M2)
            ]
            b1r = consts.tile_from(d_b1[:], name="b1r_sb")
            b2r = consts.tile_from(d_b2[:], name="b2r_sb")
            b3r = consts.tile_from(d_b3[:], name="b3r_sb")
            bcr = consts.tile_from(d_bc[:], name="bcr_sb")

            warm = ps_tp.tile([128, 4], F32, name="warm", tag="pst")
            nc.tensor.transpose(warm[0:4, 0:4], ident[0:4, 0:4], ident[0:4, 0:4])

            # natural-layout accumulators for the final combine
            a_nat = consts.tile([128, NTILE * 4], F32, name="a_nat")
            h_nat = consts.tile([128, NTILE], F32, name="h_nat")
            out_nat = consts.tile([128, NTILE], F32, name="out_nat")

            # numerical features (block 6 cols 64:77), per phase
            def emit_numcopy(p):
                t0, nt = PHASES[p]
                for tt in range(nt):
                    T = t0 + tt
                    nc.scalar.copy(
                        cnb[p][6][:, tt, 2 * EMB : KW[6]],
                        num_sb[:, T * NNUM : (T + 1) * NNUM],
                    )

            # ---------------- emission schedule ----------------
            # PE program order: p0 transposes, c0, c1, p1 tp, c2, p2 tp,
            # fin0 tp, c3, p3 tp, fin1 tp, c4, fin2-4 -- each chunk's
            # matmuls sit before the NEXT phase's transposes so compute
            # never queues behind not-yet-gathered data.
            emit_gather_phase(0)
            emit_chunk(0)
            emit_gather_phase(1)
            emit_chunk(1)
            emit_gather_phase(2)
            emit_chunk(2)
            emit_gather_phase(3)
            emit_fin(0)
            emit_chunk(3)
            emit_gather_phase(4)
            emit_fin(1)
            emit_chunk(4)
            emit_fin(2)
            emit_fin(3)
            emit_fin(4)

    nc.compile()
    return nc


_CACHE: dict = {}


def _gather_lanes(nc) -> list:
    """Per-gather DMASW lane (emission order) from the tile sem assigner."""
    import re

    gath = []
    for blk in nc.m.functions[0].blocks:
        for inst in blk.instructions:
            if type(inst).__name__ == "InstDMAGatherAnt":
                lane = None
                for u in inst.sync_info.on_update or []:
                    m = re.match(r"DMASW(\d+)_", u.ant_name or "")
                    if m:
                        lane = int(m.group(1))
                gath.append((int(inst.name.split("-")[1]), lane))
    gath.sort()
    return [lane for _, lane in gath]


def _get_nc(cross_consts) -> bass.Bass:
    """Two-pass build: the tile scheduler assigns SWDGE completion sems
    to the 8 DMASW lanes round-robin in ITS instruction order, which can
    diverge from emission order.  Each physical sem is queue-locked, so the
    gather's SWDGE queue must equal its assigned lane % 4.  Pass 1 builds
    with a nominal rotation to read the lane assignment; pass 2 rebuilds
    with queue_num = lane % 4 (queue_num doesn't affect scheduling, so the
    assignment is identical across passes)."""
    key = cross_consts
    if key not in _CACHE:
        probe = _build(cross_consts)
        qmap = [lane % 4 for lane in _gather_lanes(probe)]
        _CACHE[key] = _build(cross_consts, queue_map=qmap)
    return _CACHE[key]


def kernel(
    categorical_input,
    numerical_input,
    emb_tables,
    alphas,
    cross_bias,
    W1, b1, W2, b2, W3, b3, Wc, bc,
) -> np.ndarray:
    cat = np.ascontiguousarray(np.asarray(categorical_input, dtype=np.int64))
    num = np.ascontiguousarray(np.asarray(numerical_input, dtype=np.float32))
    emb = np.ascontiguousarray(
        np.asarray(emb_tables, dtype=np.float32).reshape(NCAT * VOCAB, EMB)
    )
    alphas = np.asarray(alphas, dtype=np.float32)
    cross_bias = np.asarray(cross_bias, dtype=np.float32)
    W1 = np.ascontiguousarray(np.asarray(W1, dtype=np.float32))
    W2 = np.ascontiguousarray(np.asarray(W2, dtype=np.float32))
    W3 = np.ascontiguousarray(np.asarray(W3, dtype=np.float32))
    Wc = np.asarray(Wc, dtype=np.float32)
    b1 = np.asarray(b1, dtype=np.float32)
    b2 = np.asarray(b2, dtype=np.float32)
    b3 = np.asarray(b3, dtype=np.float32)
    bc = np.asarray(bc, dtype=np.float32)

    # host scalar constants folding cross_bias into the per-sample chain
    cross_consts = (
        float(np.dot(alphas[1], cross_bias[0])),
        float(np.dot(alphas[2], cross_bias[0])),
        float(np.dot(alphas[2], cross_bias[1])),
        float(np.dot(Wc[:D, 0], cross_bias[0])),
        float(np.dot(Wc[:D, 0], cross_bias[1])),
        float(np.dot(Wc[:D, 0], cross_bias[2])),
    )
    nc = _get_nc(cross_consts)

    def to_dev(v):  # [D(,k)] -> [KC*128(,k)] zero-padded
        shape = (KC * 128,) + v.shape[1:]
        p = np.zeros(shape, np.float32)
        p[:D] = v
        return p

    def pad_col(v):  # [845] -> [128, KC] column-chunked, zero-padded
        return to_dev(v).reshape(KC, 128).T.copy()

    avec = np.zeros((128, KC * 4), np.float32)
    for l in range(NCROSS):
        avec[:, l::4] = pad_col(alphas[l])
    avec[:, 3::4] = pad_col(Wc[:D, 0])
    wch = Wc[D : D + L3, 0].reshape(2, 128).T.copy()
    b1r = b1.reshape(M1, 128).T.copy()
    b2r = b2.reshape(M2, 128).T.copy()
    b3r = b3.reshape(M3, 128).T.copy()
    bcr = np.broadcast_to(bc.reshape(1, 1), (128, 1)).copy()

    import ml_dtypes

    bf = ml_dtypes.bfloat16
    common = {
        "emb": emb.astype(bf),
        "w1": W1.astype(bf),
        "w2": W2.astype(bf),
        "w3": W3.astype(bf),
        "b1r": b1r,
        "b2r": b2r,
        "b3r": b3r,
        "bcr": bcr,
        "avec": avec.astype(bf),
        "wch": wch.astype(bf),
        "idf": np.eye(128, dtype=np.float32),
        "idb": np.eye(128, dtype=np.float32).astype(bf),
    }
    in_maps = []
    for core in range(NCORES):
        cs = cat[core * BC : (core + 1) * BC].astype(np.int32)  # [2048, 26]
        ns = num[core * BC : (core + 1) * BC]
        catq = np.ascontiguousarray(
            (cs & 3)
            .astype(np.int8)
            .reshape(NTILE, 128, NCAT)
            .transpose(1, 0, 2)
            .reshape(128, NTILE * NCAT)
        )
        numr = np.ascontiguousarray(
            ns.reshape(NTILE, 128, NNUM).transpose(1, 0, 2).reshape(128, NTILE * NNUM)
        )
        # gather indices: per (phase, category) block, int16 v//4,
        # lookup i at [i % 16, i // 16]; single 16-row copy (the kernel
        # replicates to the 8 partition groups on-chip)
        gi = np.zeros((16, GIDX_COLS), np.int16)
        for p, (t0, nt) in enumerate(PHASES):
            nb = nt * 128
            vs = cs[t0 * 128 : t0 * 128 + nb]  # [nb, 26]
            q4 = (vs // 4).astype(np.int16)
            wrapped = q4.reshape(nb // 16, 16, NCAT).transpose(1, 0, 2)
            for c in range(NCAT):
                blk = _PHOFF[p] + c * nt * 8
                gi[:, blk : blk + nt * 8] = wrapped[:, :, c]
        in_maps.append({**common, "catq": catq, "num": numr, "gidx": gi})

    res = run_bass_kernel_spmd(nc, in_maps, core_ids=list(range(NCORES)))
    outs = []
    for core in range(NCORES):
        o = res.results[core]["out"]  # [128, NTILE], sample T*128+p at [p, T]
        outs.append(o.T.reshape(BC, 1))
    return np.concatenate(outs, axis=0).astype(np.float32)


# revision 13
# speedup vs baseline: 1.9437x; 1.0132x over previous
"""DCN (DLRM-style deep & cross network) Trainium2 Bass kernel.

Sharding: data-parallel over batch across 8 NeuronCores (2048 samples/core).
Embedding tables (bf16) + MLP weights are replicated to every core's HBM.

Per-core pipeline (activations kept feature-major, i.e. transposed, for PE):
  1. Embedding gather via InstDMAGatherAnt: one instruction per
     (phase, category), fetching 256B quad-rows (4 bf16 vocab rows, the
     minimum 256B element) with int16 indices v//4, spread across all 4
     SWDGE queues.  Measured floor is ~2.5ns/descriptor (marginal) +
     ~1.15us fixed per instruction, so phases use the largest legal
     nidx (1024) up front and a small 256 tail: [1024, 768, 256].
  2. gidx is uploaded as a single [16, cols] tensor and replicated to
     the 8 16-partition groups by on-chip DMAs so the first gather
     launches within a few us (no 850KB serial upload in front).
  3. scalar copy + 3x copy_predicated (DVE) select the right 32-value
     quarter per lookup into the combined feature tile (no DMA-engine
     traffic besides the gathers themselves).
  4. PE 128x128 transposes -> ct[k] = combined^T chunks [128, <=512].
  5. MLP: h^T = relu(W^T @ x^T) chains, bf16 matmuls, fp32 accumulate.
  6. CrossNet folds to 4 packed dot products against x0 (alpha0..2,
     Wc_x) plus a scalar chain with host-precomputed alpha_l.b_j
     constants; final = sigmoid(x-part + Wc_h . h3 + bc).
"""

import numpy as np

import concourse.bass as bass
import concourse.mybir as mybir
import concourse.tile as tile
from concourse import bacc
from concourse.bass import broadcast_tensor_aps
from concourse.bass_utils import run_bass_kernel_spmd
F32 = mybir.dt.float32
BF16 = mybir.dt.bfloat16
I32 = mybir.dt.int32
I16 = mybir.dt.int16
I8 = mybir.dt.int8

B = 16384
NCORES = 8
BC = B // NCORES            # 2048 samples per core
NCAT = 26
VOCAB = 100000
EMB = 32
NNUM = 13
D = NCAT * EMB + NNUM       # 845

L1, L2, L3 = 1024, 512, 256
NCROSS = 3
KC = 7                      # feature chunks of 128 (6*128 + 77)
KW = [128] * 6 + [D - 6 * 128]
CPB = 128 // EMB            # 4 categories per 128-feature block
M1, M2, M3 = L1 // 128, L2 // 128, L3 // 128   # 8, 4, 2
NTILE = BC // 128           # 16 batch tiles per core

# gather phases: (first tile, #tiles); nidx = #tiles * 128 (max 1024).
# Front-loaded 1024 phase amortizes the ~1.15us/instruction fixed cost;
# the small late phases keep the post-gather compute tail short.
PHASES = [(0, 4), (4, 4), (8, 4), (12, 2), (14, 2)]
NPH = len(PHASES)
# compute chunks: (first tile, #tiles, phase fed by)
CHUNKS = [(0, 4, 0), (4, 4, 1), (8, 4, 2), (12, 2, 3), (14, 2, 4)]
NCHUNK = len(CHUNKS)
# idx column offset (in 16-partition-wrapped cols) per phase block
_PHOFF = [0]
for _t0, _nt in PHASES:
    _PHOFF.append(_PHOFF[-1] + NCAT * _nt * 8)
GIDX_COLS = _PHOFF[-1]      # 3328 total
_P0COLS = _PHOFF[1]         # phase-0 block


def _build(cross_consts, queue_map=None) -> bass.Bass:
    # cross_consts = (c10, c20, c21, d0, d1, d2):
    #   c_lj = alpha_l . cross_bias_j,  d_j = Wc_x . cross_bias_j
    c10, c20, c21, d0, d1, d2 = cross_consts

    nc = bacc.Bacc("TRN2", target_bir_lowering=False, num_swdge_queues=4)

    d_emb = nc.dram_tensor("emb", [NCAT * VOCAB, EMB], BF16, kind="ExternalInput")
    d_catq = nc.dram_tensor("catq", [128, NTILE * NCAT], I8, kind="ExternalInput")
    d_gidx = nc.dram_tensor("gidx", [16, GIDX_COLS], I16, kind="ExternalInput")
    d_num = nc.dram_tensor("num", [128, NTILE * NNUM], F32, kind="ExternalInput")
    d_w1 = nc.dram_tensor("w1", [D, L1], BF16, kind="ExternalInput")
    d_w2 = nc.dram_tensor("w2", [L1, L2], BF16, kind="ExternalInput")
    d_w3 = nc.dram_tensor("w3", [L2, L3], BF16, kind="ExternalInput")
    d_b1 = nc.dram_tensor("b1r", [128, M1], F32, kind="ExternalInput")
    d_b2 = nc.dram_tensor("b2r", [128, M2], F32, kind="ExternalInput")
    d_b3 = nc.dram_tensor("b3r", [128, M3], F32, kind="ExternalInput")
    d_bc = nc.dram_tensor("bcr", [128, 1], F32, kind="ExternalInput")
    # avec: per k-chunk 4 columns [alpha0, alpha1, alpha2, wc_x]
    d_avec = nc.dram_tensor("avec", [128, KC * 4], BF16, kind="ExternalInput")
    d_idf = nc.dram_tensor("idf", [128, 128], F32, kind="ExternalInput")
    d_idb = nc.dram_tensor("idb", [128, 128], BF16, kind="ExternalInput")
    d_wch = nc.dram_tensor("wch", [128, 2], BF16, kind="ExternalInput")
    d_out = nc.dram_tensor("out", [128, NTILE], F32, kind="ExternalOutput")

    with tile.TileContext(nc) as tc:
        with (
            tc.tile_pool(name="consts", bufs=1) as consts,
            tc.tile_pool(name="quadp", bufs=13) as quadp,
            tc.tile_pool(name="quadp2", bufs=26) as quadp2,
            tc.tile_pool(name="ctp", bufs=2) as ctp,
            tc.tile_pool(name="actp", bufs=3) as actp,
            tc.tile_pool(name="rowp", bufs=1) as rowp,
            tc.tile_pool(name="ps_mm", bufs=3, space="PSUM") as ps_mm,
            tc.tile_pool(name="ps_tp", bufs=2, space="PSUM") as ps_tp,
            tc.tile_pool(name="ps_a", bufs=2, space="PSUM") as ps_a,
            tc.tile_pool(name="ps_h", bufs=1, space="PSUM") as ps_h,
        ):
            # --------- minimal-latency index upload, then gathers ---------
            # catq (cat & 3, int8, 53KB) first: masks depend on it.
            catq = consts.tile([128, NTILE * NCAT], I8, name="catq_sb")
            nc.sync.dma_start(catq[:], d_catq[:])
            # gidx replicated on-chip: phase-0 cols by sync engine,
            # the rest by the scalar engine, 8 partition-groups each.
            # gidx in two tiles so phase-0 gathers only dep on the
            # phase-0 replicas; those 8 replica DMAs dispatch from two
            # queues (sync+scalar) in parallel.
            gidx0 = consts.tile([128, _P0COLS], I16, name="gidx0_sb")
            gidx1 = consts.tile([128, GIDX_COLS - _P0COLS], I16, name="gidx1_sb")
            for g in range(8):
                eng = nc.sync if g % 2 == 0 else nc.scalar
                eng.dma_start(
                    gidx0[g * 16 : (g + 1) * 16, :], d_gidx[:, 0:_P0COLS]
                )
            # identities next on the sync queue: first PE transpose needs
            # ident_bf at ~10us
            ident_bf = consts.tile_from(d_idb[:], name="ident_bf")
            ident = consts.tile_from(d_idf[:], name="ident")
            for g in range(8):
                nc.sync.dma_start(
                    gidx1[g * 16 : (g + 1) * 16, :], d_gidx[:, _P0COLS:]
                )

            # quarter-select masks: m_i = ((cat & 3) == i), i in 1..3
            masks = []
            for i in range(1, 4):
                mi = consts.tile([128, NTILE * NCAT], I8, name=f"m{i}")
                nc.vector.tensor_single_scalar(
                    mi[:], catq[:], i, mybir.AluOpType.is_equal
                )
                masks.append(mi[:].rearrange("p (T c) -> p T c", c=NCAT))

            # combined features per (phase, k-block), batch-natural bf16
            cnb = [
                [
                    consts.tile([128, PHASES[p][1], KW[k]], BF16, name=f"cnb{p}_{k}")
                    for k in range(KC)
                ]
                for p in range(NPH)
            ]

            _g = [0]  # global gather counter for queue assignment

            def emit_gather_phase(p):
                """Per category: SWDGE gather, then quarter-select into cnb
                (scalar copy + 3 DVE predicated copies), then PE transposes
                for each completed 4-category k-block. The numerical-feature
                copies are emitted just before c=24 so the scalar queue isn't
                head-of-line blocked on the num upload early in the phase."""
                t0, nt = PHASES[p]
                nidx = nt * 128
                pool = quadp2 if nt <= 2 else quadp
                for c in range(NCAT):
                    if c == 24:
                        emit_numcopy(p)
                    quad = pool.tile([128, nt, 4 * EMB], BF16, name=f"quad{nt}",
                                     tag=f"quad{nt}")
                    nc.gpsimd.dma_gather(
                        out_ap=quad[:],
                        in_ap=d_emb[c * VOCAB : (c + 1) * VOCAB, :].rearrange(
                            "(r q) e -> r (q e)", q=4
                        ),
                        idxs_ap=(gidx0 if p == 0 else gidx1)[
                            :,
                            _PHOFF[p] - (0 if p == 0 else _P0COLS)
                            + c * nt * 8 : _PHOFF[p]
                            - (0 if p == 0 else _P0COLS)
                            + (c + 1) * nt * 8,
                        ],
                        num_idxs=nidx,
                        num_idxs_reg=nidx,
                        elem_size=4 * EMB,
                        queue_num=(
                            queue_map[_g[0]] if queue_map else _g[0] % 4
                        ),
                    )
                    _g[0] += 1
                    co = (c % CPB) * EMB
                    dest = cnb[p][c // CPB][:, :, co : co + EMB]
                    # e0 path via sync-queue SBUF->SBUF DMA: keeps the
                    # scalar queue free for activations (PSUM recycling
                    # feeds the PE) and off the gather-dependent path.
                    nc.sync.dma_start(dest, quad[:, :, 0:EMB])
                    for i in range(1, 4):
                        mslice = masks[i - 1][:, t0 : t0 + nt, c : c + 1]
                        mb, _ = broadcast_tensor_aps(mslice, dest)
                        nc.vector.copy_predicated(
                            dest, mb, quad[:, :, i * EMB : (i + 1) * EMB]
                        )
                    if c % CPB == CPB - 1 and c // CPB < 6:
                        emit_transposes(p, c // CPB)
                    if c == NCAT - 1:
                        emit_transposes(p, 6)

            ct_tiles = {}
            psa_tiles = {}
            psh_tiles = {}

            def emit_transposes(p, k):
                kw = KW[k]
                pt0, pnt = PHASES[p]
                for ci, (t0, nt, cp) in enumerate(CHUNKS):
                    if cp != p:
                        continue
                    ctk = ctp.tile([128, 512], BF16, name=f"ct{k}", tag=f"ct{k}")
                    ct_tiles[(ci, k)] = ctk
                    for t in range(nt):
                        tt = (t0 - pt0) + t
                        pst = ps_tp.tile([128, 128], BF16, name="pst", tag="pst")
                        nc.tensor.transpose(
                            pst[0:kw, :],
                            cnb[p][k][:, tt, 0:kw],
                            ident_bf[:],
                        )
                        nc.any.tensor_copy(
                            ctk[0:kw, t * 128 : (t + 1) * 128], pst[0:kw, :]
                        )

            def emit_chunk(ci):
                t0, nt, cp = CHUNKS[ci]
                w = nt * 128
                ct = [ct_tiles[(ci, k)] for k in range(KC)]
                # cross-net dot products: [alpha0, alpha1, alpha2, wc_x]
                psa = ps_a.tile([4, 512], F32, name="psa", tag="psa")
                for k in range(KC):
                    kw = KW[k]
                    nc.tensor.matmul(
                        psa[:, 0:w],
                        avec[0:kw, k * 4 : (k + 1) * 4],
                        ct[k][0:kw, 0:w],
                        start=(k == 0),
                        stop=(k == KC - 1),
                    )
                # MLP
                h1 = []
                for m in range(M1):
                    psm = ps_mm.tile([128, 512], F32, name="psm")
                    for k in range(KC):
                        kw = KW[k]
                        nc.tensor.matmul(
                            psm[:, 0:w],
                            w1[k][0:kw, m * 128 : (m + 1) * 128],
                            ct[k][0:kw, 0:w],
                            start=(k == 0),
                            stop=(k == KC - 1),
                        )
                    h = actp.tile([128, 512], BF16, name=f"h1_{m}", tag=f"h1_{m}")
                    nc.scalar.activation(
                        h[:, 0:w], psm[:, 0:w], mybir.ActivationFunctionType.Relu,
                        bias=b1r[:, m : m + 1],
                    )
                    h1.append(h)
                h2 = []
                for m in range(M2):
                    psm = ps_mm.tile([128, 512], F32, name="psm")
                    for k in range(M1):
                        nc.tensor.matmul(
                            psm[:, 0:w],
                            w2[k][:, m * 128 : (m + 1) * 128],
                            h1[k][:, 0:w],
                            start=(k == 0),
                            stop=(k == M1 - 1),
                        )
                    h = actp.tile([128, 512], BF16, name=f"h2_{m}", tag=f"h2_{m}")
                    nc.scalar.activation(
                        h[:, 0:w], psm[:, 0:w], mybir.ActivationFunctionType.Relu,
                        bias=b2r[:, m : m + 1],
                    )
                    h2.append(h)
                h3 = []
                for m in range(M3):
                    psm = ps_mm.tile([128, 512], F32, name="psm")
                    for k in range(M2):
                        nc.tensor.matmul(
                            psm[:, 0:w],
                            w3[k][:, m * 128 : (m + 1) * 128],
                            h2[k][:, 0:w],
                            start=(k == 0),
                            stop=(k == M2 - 1),
                        )
                    h = actp.tile([128, 512], BF16, name=f"h3_{m}", tag=f"h3_{m}")
                    nc.scalar.activation(
                        h[:, 0:w], psm[:, 0:w],
                        mybir.ActivationFunctionType.Identity,
                        bias=b3r[:, m : m + 1],
                    )
                    h3.append(h)

                # h3 . wc_h -> row
                psh = ps_h.tile([1, 512], F32, name="psh", tag="psrow")
                for j in range(M3):
                    nc.tensor.matmul(
                        psh[:, 0:w], wch[:, j : j + 1], h3[j][:, 0:w],
                        start=(j == 0), stop=(j == M3 - 1),
                    )
                psa_tiles[ci] = psa
                psh_tiles[ci] = psh

            # ------------- final combine (batch-natural, per chunk) --------
            # x3 = p3*x0 + q30*b0 + q31*b1 + b2 with per-sample scalars from
            # the a-dots; Wc_x.x3 folds to p3*awc + q30*d0 + q31*d1 + d2.
            def emit_fin(ci):
                t0, nt, cp = CHUNKS[ci]
                w = nt * 128
                a_sb = actp.tile([4, 512], F32, name="a_sb", tag="a_sb")
                nc.any.tensor_copy(a_sb[:, 0:w], psa_tiles[ci][:, 0:w])
                h_sb = actp.tile([1, 512], F32, name="h_sb", tag="h_sb")
                nc.any.tensor_copy(h_sb[:, 0:w], psh_tiles[ci][:, 0:w])
                for t in range(nt):
                    pta = ps_tp.tile([128, 4], F32, name="pta", tag="pst")
                    nc.tensor.transpose(
                        pta[:], a_sb[:, t * 128 : (t + 1) * 128], ident[0:4, 0:4]
                    )
                    T = t0 + t
                    nc.vector.tensor_copy(a_nat[:, T * 4 : (T + 1) * 4], pta[:])
                    pth = ps_tp.tile([128, 1], F32, name="pth", tag="pst")
                    nc.tensor.transpose(
                        pth[:], h_sb[:, t * 128 : (t + 1) * 128], ident[0:1, 0:1]
                    )
                    nc.vector.tensor_copy(h_nat[:, T : T + 1], pth[:])
                av = a_nat[:, t0 * 4 : (t0 + nt) * 4].rearrange(
                    "p (t l) -> p t l", l=4
                )
                a0, a1, a2, awc = (av[:, :, l] for l in range(4))
                hn = h_nat[:, t0 : t0 + nt]

                def rtile(name):
                    return rowp.tile([128, nt], F32, name=name, tag=f"{name}_{ci}")

                p1 = rtile("p1")            # 1 + s0
                nc.vector.tensor_scalar_add(p1[:], a0, 1.0)
                s1 = rtile("s1")            # s1 = p1*a1 (+ c10)
                nc.vector.tensor_mul(s1[:], a1, p1[:])
                if c10 != 0.0:
                    nc.vector.tensor_scalar_add(s1[:], s1[:], float(c10))
                u1 = rtile("u1")            # 1 + s1  (= q20)
                nc.vector.tensor_scalar_add(u1[:], s1[:], 1.0)
                p2 = rtile("p2")
                nc.vector.tensor_mul(p2[:], p1[:], u1[:])
                s2 = rtile("s2")            # s2 = p2*a2 + u1*c20 + c21
                nc.vector.tensor_mul(s2[:], a2, p2[:])
                if c20 != 0.0:
                    v20 = rtile("v20")
                    nc.vector.tensor_scalar_mul(v20[:], u1[:], float(c20))
                    nc.vector.tensor_add(s2[:], s2[:], v20[:])
                if c21 != 0.0:
                    nc.vector.tensor_scalar_add(s2[:], s2[:], float(c21))
                u2 = rtile("u2")            # 1 + s2
                nc.vector.tensor_scalar_add(u2[:], s2[:], 1.0)
                p3 = rtile("p3")
                nc.vector.tensor_mul(p3[:], p2[:], u2[:])
                fin = rtile("fin")          # awc*p3 (+ bias-derived terms)
                nc.vector.tensor_mul(fin[:], awc, p3[:])
                if d0 != 0.0:
                    q30 = rtile("q30")
                    nc.vector.tensor_mul(q30[:], u1[:], u2[:])
                    nc.vector.tensor_scalar_mul(q30[:], q30[:], float(d0))
                    nc.vector.tensor_add(fin[:], fin[:], q30[:])
                if d1 != 0.0:
                    w1t = rtile("w1t")
                    nc.vector.tensor_scalar_mul(w1t[:], u2[:], float(d1))
                    nc.vector.tensor_add(fin[:], fin[:], w1t[:])
                if d2 != 0.0:
                    nc.vector.tensor_scalar_add(fin[:], fin[:], float(d2))
                nc.vector.tensor_add(fin[:], fin[:], hn)
                ons = out_nat[:, t0 : t0 + nt]
                nc.scalar.activation(
                    ons, fin[:], mybir.ActivationFunctionType.Sigmoid,
                    bias=bcr[:, 0:1],
                )
                nc.sync.dma_start(d_out[:, t0 : t0 + nt], ons)

            # constants / weights (upload overlaps the gather stream)
            num_sb = consts.tile([128, NTILE * NNUM], F32, name="num_sb")
            nc.sync.dma_start(num_sb[:], d_num[:])
            avec = consts.tile_from(d_avec[:], name="avec_sb")
            wch = consts.tile_from(d_wch[:], name="wch_sb")
            w1 = [
                consts.tile_from(d_w1[k * 128 : k * 128 + KW[k], :], name=f"w1_{k}")
                for k in range(KC)
            ]
            w2 = [
                consts.tile_from(d_w2[k * 128 : (k + 1) * 128, :], name=f"w2_{k}")
                for k in range(M1)
            ]
            w3 = [
                consts.tile_from(d_w3[k * 128 : (k + 1) * 128, :], name=f"w3_{k}")
                for k in range(M2)
            ]
            b1r = consts.tile_from(d_b1[:], name="b1r_sb")
            b2r = consts.tile_from(d_b2[:], name="b2r_sb")
            b3r = consts.tile_from(d_b3[:], name="b3r_sb")
            bcr = consts.tile_from(d_bc[:], name="bcr_sb")

            warm = ps_tp.tile([128, 4], F32, name="warm", tag="pst")
            nc.tensor.transpose(warm[0:4, 0:4], ident[0:4, 0:4], ident[0:4, 0:4])

            # natural-layout accumulators for the final combine
            a_nat = consts.tile([128, NTILE * 4], F32, name="a_nat")
            h_nat = consts.tile([128, NTILE], F32, name="h_nat")
            out_nat = consts.tile([128, NTILE], F32, name="out_nat")

            # numerical features (block 6 cols 64:77), per phase
            def emit_numcopy(p):
                t0, nt = PHASES[p]
                for tt in range(nt):
                    T = t0 + tt
                    nc.scalar.copy(
                        cnb[p][6][:, tt, 2 * EMB : KW[6]],
                        num_sb[:, T * NNUM : (T + 1) * NNUM],
                    )

            # ---------------- emission schedule ----------------
            # PE program order: p0 transposes, c0, c1, p1 tp, c2, p2 tp,
            # fin0 tp, c3, p3 tp, fin1 tp, c4, fin2-4 -- each chunk's
            # matmuls sit before the NEXT phase's transposes so compute
            # never queues behind not-yet-gathered data.
            emit_gather_phase(0)
            emit_chunk(0)
            emit_gather_phase(1)
            emit_chunk(1)
            emit_gather_phase(2)
            emit_chunk(2)
            emit_gather_phase(3)
            emit_fin(0)
            emit_chunk(3)
            emit_gather_phase(4)
            emit_fin(1)
            emit_chunk(4)
            emit_fin(2)
            emit_fin(3)
            emit_fin(4)

    nc.compile()
    return nc


_CACHE: dict = {}


def _gather_lanes(nc) -> list:
    """Per-gather DMASW lane (emission order) from the tile sem assigner."""
    import re

    gath = []
    for blk in nc.m.functions[0].blocks:
        for inst in blk.instructions:
            if type(inst).__name__ == "InstDMAGatherAnt":
                lane = None
                for u in inst.sync_info.on_update or []:
                    m = re.match(r"DMASW(\d+)_", u.ant_name or "")
                    if m:
                        lane = int(m.group(1))
                gath.append((int(inst.name.split("-")[1]), lane))
    gath.sort()
    return [lane for _, lane in gath]


def _get_nc(cross_consts) -> bass.Bass:
    """Two-pass build: the tile scheduler assigns SWDGE completion sems
    to the 8 DMASW lanes round-robin in ITS instruction order, which can
    diverge from emission order.  Each physical sem is queue-locked, so the
    gather's SWDGE queue must equal its assigned lane % 4.  Pass 1 builds
    with a nominal rotation to read the lane assignment; pass 2 rebuilds
    with queue_num = lane % 4 (queue_num doesn't affect scheduling, so the
    assignment is identical across passes)."""
    key = cross_consts
    if key not in _CACHE:
        probe = _build(cross_consts)
        qmap = [lane % 4 for lane in _gather_lanes(probe)]
        _CACHE[key] = _build(cross_consts, queue_map=qmap)
    return _CACHE[key]


def kernel(
    categorical_input,
    numerical_input,
    emb_tables,
    alphas,
    cross_bias,
    W1, b1, W2, b2, W3, b3, Wc, bc,
) -> np.ndarray:
    cat = np.ascontiguousarray(np.asarray(categorical_input, dtype=np.int64))
    num = np.ascontiguousarray(np.asarray(numerical_input, dtype=np.float32))
    emb = np.ascontiguousarray(
        np.asarray(emb_tables, dtype=np.float32).reshape(NCAT * VOCAB, EMB)
    )
    alphas = np.asarray(alphas, dtype=np.float32)
    cross_bias = np.asarray(cross_bias, dtype=np.float32)
    W1 = np.ascontiguousarray(np.asarray(W1, dtype=np.float32))
    W2 = np.ascontiguousarray(np.asarray(W2, dtype=np.float32))
    W3 = np.ascontiguousarray(np.asarray(W3, dtype=np.float32))
    Wc = np.asarray(Wc, dtype=np.float32)
    b1 = np.asarray(b1, dtype=np.float32)
    b2 = np.asarray(b2, dtype=np.float32)
    b3 = np.asarray(b3, dtype=np.float32)
    bc = np.asarray(bc, dtype=np.float32)

    # host scalar constants folding cross_bias into the per-sample chain
    cross_consts = (
        float(np.dot(alphas[1], cross_bias[0])),
        float(np.dot(alphas[2], cross_bias[0])),
        float(np.dot(alphas[2], cross_bias[1])),
        float(np.dot(Wc[:D, 0], cross_bias[0])),
        float(np.dot(Wc[:D, 0], cross_bias[1])),
        float(np.dot(Wc[:D, 0], cross_bias[2])),
    )
    nc = _get_nc(cross_consts)

    def to_dev(v):  # [D(,k)] -> [KC*128(,k)] zero-padded
        shape = (KC * 128,) + v.shape[1:]
        p = np.zeros(shape, np.float32)
        p[:D] = v
        return p

    def pad_col(v):  # [845] -> [128, KC] column-chunked, zero-padded
        return to_dev(v).reshape(KC, 128).T.copy()

    avec = np.zeros((128, KC * 4), np.float32)
    for l in range(NCROSS):
        avec[:, l::4] = pad_col(alphas[l])
    avec[:, 3::4] = pad_col(Wc[:D, 0])
    wch = Wc[D : D + L3, 0].reshape(2, 128).T.copy()
    b1r = b1.reshape(M1, 128).T.copy()
    b2r = b2.reshape(M2, 128).T.copy()
    b3r = b3.reshape(M3, 128).T.copy()
    bcr = np.broadcast_to(bc.reshape(1, 1), (128, 1)).copy()

    import ml_dtypes

    bf = ml_dtypes.bfloat16
    common = {
        "emb": emb.astype(bf),
        "w1": W1.astype(bf),
        "w2": W2.astype(bf),
        "w3": W3.astype(bf),
        "b1r": b1r,
        "b2r": b2r,
        "b3r": b3r,
        "bcr": bcr,
        "avec": avec.astype(bf),
        "wch": wch.astype(bf),
        "idf": np.eye(128, dtype=np.float32),
        "idb": np.eye(128, dtype=np.float32).astype(bf),
    }
    in_maps = []
    for core in range(NCORES):
        cs = cat[core * BC : (core + 1) * BC].astype(np.int32)  # [2048, 26]
        ns = num[core * BC : (core + 1) * BC]
        catq = np.ascontiguousarray(
            (cs & 3)
            .astype(np.int8)
            .reshape(NTILE, 128, NCAT)
            .transpose(1, 0, 2)
            .reshape(128, NTILE * NCAT)
        )
        numr = np.ascontiguousarray(
            ns.reshape(NTILE, 128, NNUM).transpose(1, 0, 2).reshape(128, NTILE * NNUM)
        )
        # gather indices: per (phase, category) block, int16 v//4,
        # lookup i at [i % 16, i // 16]; single 16-row copy (the kernel
        # replicates to the 8 partition groups on-chip)
        gi = np.zeros((16, GIDX_COLS), np.int16)
        for p, (t0, nt) in enumerate(PHASES):
            nb = nt * 128
            vs = cs[t0 * 128 : t0 * 128 + nb]  # [nb, 26]
            q4 = (vs // 4).astype(np.int16)
            wrapped = q4.reshape(nb // 16, 16, NCAT).transpose(1, 0, 2)
            for c in range(NCAT):
                blk = _PHOFF[p] + c * nt * 8
                gi[:, blk : blk + nt * 8] = wrapped[:, :, c]
        in_maps.append({**common, "catq": catq, "num": numr, "gidx": gi})

    res = run_bass_kernel_spmd(nc, in_maps, core_ids=list(range(NCORES)))
    outs = []
    for core in range(NCORES):
        o = res.results[core]["out"]  # [128, NTILE], sample T*128+p at [p, T]
        outs.append(o.T.reshape(BC, 1))
    return np.concatenate(outs, axis=0).astype(np.float32)
